# revision 1
# baseline (speedup 1.0000x reference)
"""Attention-LSTM decoder (nn_Decoder) Trainium2 Bass kernel.

Sharding: data-parallel over batch B=64 -> 8 cores x 8 examples.
Per core, everything (K^T, V, weights) is SBUF-resident; the T=200 step
recurrence is fully unrolled.

Per-step dataflow (per core, BL=8 local examples):
  gates  : PE stream MMs  psum[8,512] x4 chunks  (lhsT = featT [128,8] bf16,
           rhs = W^T [128,512] bf16, + f32 bias MM via ones-row)
  lstm   : ACT sigmoid/tanh from psum, DVE muls/adds, f32 states
  hT     : PE transpose [8,128]->[128,8] x4, copy to bf16
  energy : PE stream MMs out [1,448] x2 halves per ex (lhsT = hT col [128,1],
           rhs = K^T tile [128,448] bf16), exp via ACT (scale=1/sqrt(128))
  softmax: no max-subtraction (energies are small); fused mask-mult + row-sum
           (DVE tensor_tensor_reduce), reciprocal, per-row normalize to bf16
  attnT  : PE transpose [8,128]->[128,8] x7, copy bf16
  ctx    : PE weights-path MMs out [128c,1] accumulate over 7 s-chunks
           (lhsT = V tile [128s,128c] bf16 FWL, rhs = attnT col [128,1])
  mlp    : PE stream MMs [8,512] + tanh, transpose, logits [8,34], DMA out
"""

import math
import os
import sys

import numpy as np

sys.path.insert(0, "/opt/trn_rl_repo")

import ml_dtypes  # noqa: E402

import concourse.bass as bass  # noqa: E402
import concourse.bacc as bacc  # noqa: E402
import concourse.tile as tile  # noqa: E402
from concourse import mybir  # noqa: E402
from concourse.masks import make_identity  # noqa: E402

BF16 = ml_dtypes.bfloat16

V, E, H, C = 34, 256, 512, 512
B, T, S = 64, 200, 800
SP = 896  # padded S (7 * 128)
NCORES = 8
BL = B // NCORES  # 8 examples per core
SCALE = 1.0 / math.sqrt(128.0)
FDT = mybir.dt.float32
BDT = mybir.dt.bfloat16


def build_bass(t_steps: int = T) -> bass.Bass:
    nc = bacc.Bacc()

    kts_d = nc.dram_tensor("kts", [BL, 4, 128, SP], BDT, kind="ExternalInput")
    v_d = nc.dram_tensor("v", [BL, 7, 128, C], BDT, kind="ExternalInput")
    w_d = nc.dram_tensor("wt", [10, 128, 4 * H], BDT, kind="ExternalInput")
    w1_d = nc.dram_tensor("w1t", [8, 128, C], BDT, kind="ExternalInput")
    w2_d = nc.dram_tensor("w2t", [4, 128, V], BDT, kind="ExternalInput")
    xemb_d = nc.dram_tensor("xembt", [t_steps, 128, 2, BL], BDT, kind="ExternalInput")
    mask_d = nc.dram_tensor("mask", [128, 7, BL], FDT, kind="ExternalInput")
    brow_d = nc.dram_tensor("brow", [1, 4 * H], FDT, kind="ExternalInput")
    b1_d = nc.dram_tensor("b1row", [1, C], FDT, kind="ExternalInput")
    b2_d = nc.dram_tensor("b2row", [1, V], FDT, kind="ExternalInput")
    out_d = nc.dram_tensor("out", [t_steps, BL, V], FDT, kind="ExternalOutput")

    from contextlib import ExitStack

    with tile.TileContext(nc) as tc, ExitStack() as es:
        consts = es.enter_context(tc.tile_pool(name="consts", bufs=1))
        state = es.enter_context(tc.tile_pool(name="state", bufs=1))
        work = es.enter_context(tc.tile_pool(name="work", bufs=1))
        xpool = es.enter_context(tc.tile_pool(name="xpool", bufs=4))
        opool = es.enter_context(tc.tile_pool(name="opool", bufs=4))
        pg = es.enter_context(tc.tile_pool(name="pg", bufs=2, space="PSUM"))
        pe_ = es.enter_context(tc.tile_pool(name="pe", bufs=2, space="PSUM"))
        pc = es.enter_context(tc.tile_pool(name="pc", bufs=2, space="PSUM"))
        pt = es.enter_context(tc.tile_pool(name="pt", bufs=2, space="PSUM"))

        # ---- resident tensors ----
        kts_sb = consts.tile([128, BL, 4, SP], BDT)
        for ex in range(BL):
            for hk in range(4):
                nc.sync.dma_start(out=kts_sb[:, ex, hk, :], in_=kts_d[ex, hk])
        v_sb = consts.tile([128, BL, 7, C], BDT)
        for ex in range(BL):
            for sj in range(7):
                nc.sync.dma_start(out=v_sb[:, ex, sj, :], in_=v_d[ex, sj])
        w_sb = consts.tile([128, 10, 4 * H], BDT)
        for k in range(10):
            nc.sync.dma_start(out=w_sb[:, k, :], in_=w_d[k])
        w1_sb = consts.tile([128, 8, C], BDT)
        for k in range(8):
            nc.sync.dma_start(out=w1_sb[:, k, :], in_=w1_d[k])
        w2_sb = consts.tile([128, 4, V], BDT)
        for k in range(4):
            nc.sync.dma_start(out=w2_sb[:, k, :], in_=w2_d[k])
        maskc_sb = consts.tile([128, 7, BL], FDT)
        nc.sync.dma_start(out=maskc_sb, in_=mask_d[:, :, :])
        brow_sb = consts.tile([1, 4 * H], FDT)
        nc.sync.dma_start(out=brow_sb, in_=brow_d[:, :])
        b1_sb = consts.tile([1, C], FDT)
        nc.sync.dma_start(out=b1_sb, in_=b1_d[:, :])
        b2_sb = consts.tile([1, V], FDT)
        nc.sync.dma_start(out=b2_sb, in_=b2_d[:, :])
        ones8 = consts.tile([1, BL], FDT)
        nc.vector.memset(ones8, 1.0)
        ones128c = consts.tile([128, 1], FDT)
        nc.vector.memset(ones128c, 1.0)
        ones128r = consts.tile([1, 128], FDT)
        nc.vector.memset(ones128r, 1.0)
        id8f = consts.tile([BL, BL], FDT)
        make_identity(nc, id8f)

        # ---- recurrent state ----
        hT = state.tile([128, 4, BL], BDT)  # h^T, feature-on-partition
        ctxT = state.tile([128, 4, BL], BDT)  # ctx^T
        cst = state.tile([BL, H], FDT)  # cell state, row layout
        nc.vector.memset(hT, 0.0)
        nc.vector.memset(ctxT, 0.0)
        nc.vector.memset(cst, 0.0)

        AF = mybir.ActivationFunctionType
        OP = mybir.AluOpType

        for t in range(t_steps):
            xT = xpool.tile([128, 2, BL], BDT)
            nc.sync.dma_start(out=xT, in_=xemb_d[t])

            def feat(k):
                if k < 2:
                    return xT[:, k, :]
                if k < 6:
                    return ctxT[:, k - 2, :]
                return hT[:, k - 6, :]

            # ---- LSTM gates: psum [8,512] per gate ----
            gact = []  # sbuf tiles: sig_i, sig_f, tanh_g, sig_o
            for n in range(4):
                g_ps = pg.tile([BL, 512], FDT, tag="g")
                for k in range(10):
                    nc.tensor.matmul(
                        g_ps,
                        feat(k),
                        w_sb[:, k, n * 512 : (n + 1) * 512],
                        start=(k == 0),
                        stop=False,
                    )
                nc.tensor.matmul(
                    g_ps,
                    ones8,
                    brow_sb[:, n * 512 : (n + 1) * 512],
                    start=False,
                    stop=True,
                )
                ga = work.tile([BL, 512], FDT, tag=f"ga{n}")
                nc.scalar.activation(ga, g_ps, AF.Tanh if n == 2 else AF.Sigmoid)
                gact.append(ga)

            # tensor_tensor_reduce instead of tensor_tensor: the plain TT ISA
            # struct only has one sync-wait slot (codegen rejects 2+ waits);
            # TTR lowers to an ISA struct with enough slots. accum is unused.
            def tt(out, a, b, op):
                nc.vector.scalar_tensor_tensor(
                    out=out, in0=a, scalar=1.0, in1=b, op0=OP.mult, op1=op
                )

            # keep every 2-tensor DVE op's inputs produced by ACT only (the
            # DVE 2-src ISA structs have a single sync-wait slot): bounce the
            # cell state through an ACT copy each step.
            c_act = work.tile([BL, H], FDT, tag="c_act")
            nc.scalar.copy(c_act, cst)
            t_ig = work.tile([BL, H], FDT, tag="t_ig")
            tt(t_ig, gact[0], gact[2], OP.mult)
            t_fc = work.tile([BL, H], FDT, tag="t_fc")
            tt(t_fc, gact[1], c_act, OP.mult)
            c_new = work.tile([BL, H], FDT, tag=f"c{t % 2}")
            tt(c_new, t_ig, t_fc, OP.add)
            cst = c_new
            tnc = work.tile([BL, H], FDT, tag="tnc")
            nc.scalar.activation(tnc, cst, AF.Tanh)
            h_sb = work.tile([BL, H], FDT, tag="h_sb")
            tt(h_sb, gact[3], tnc, OP.mult)

            # ---- h^T (bf16) via PE transpose ----
            for ck in range(4):
                tp = pt.tile([128, BL], FDT, tag="t")
                nc.tensor.transpose(tp, h_sb[:, ck * 128 : (ck + 1) * 128], id8f)
                nc.vector.tensor_copy(hT[:, ck, :], tp)

            # ---- attention: energy column-form [128s, 7] per example ----
            # e[s] = sum_h K^T[h, s] * h[h]; exp(scale*e); mask-mult with
            # fused per-partition partial sums; partition-sum via ones-MM.
            expm = work.tile([128, 7, BL], FDT, tag="expm")
            amb = work.tile([128, 7, BL], BDT, tag="amb")
            partials = work.tile([128, BL], FDT, tag="partials")
            for ex in range(BL):
                e_ps = pe_.tile([128, 7], FDT, tag="e")
                for sj in range(7):
                    for hk in range(4):
                        nc.tensor.matmul(
                            e_ps[:, sj : sj + 1],
                            kts_sb[:, ex, hk, sj * 128 : (sj + 1) * 128],
                            hT[:, hk, ex : ex + 1],
                            start=(hk == 0),
                            stop=(hk == 3),
                        )
                nc.scalar.activation(expm[:, :, ex], e_ps, AF.Exp, scale=SCALE)
                nc.vector.scalar_tensor_tensor(
                    out=amb[:, :, ex],
                    in0=expm[:, :, ex],
                    scalar=1.0,
                    in1=maskc_sb[:, :, ex],
                    op0=OP.mult,
                    op1=OP.mult,
                    accum_out=partials[:, ex : ex + 1],
                )

            es_ps = pt.tile([1, BL], FDT, tag="t")
            nc.tensor.matmul(es_ps, ones128c, partials, start=True, stop=True)
            rinv8 = work.tile([1, BL], FDT, tag="rinv8")
            nc.vector.reciprocal(rinv8, es_ps)
            rb_ps = pt.tile([128, BL], FDT, tag="t")
            nc.tensor.matmul(rb_ps, ones128r, rinv8, start=True, stop=True)
            rb_sb = work.tile([128, BL], FDT, tag="rb_sb")
            nc.scalar.copy(rb_sb, rb_ps)
            anb = work.tile([128, 7, BL], BDT, tag="anb")
            for ex in range(BL):
                nc.vector.tensor_scalar_mul(
                    anb[:, :, ex], amb[:, :, ex], rb_sb[:, ex : ex + 1]
                )

            # ---- context: ctx^T[128c, ex] accumulated over 7 s-chunks ----
            for ex in range(BL):
                c_ps = pc.tile([128, 4], FDT, tag="c")
                for ck in range(4):
                    for sj in range(7):
                        nc.tensor.matmul(
                            c_ps[:, ck : ck + 1],
                            v_sb[:, ex, sj, ck * 128 : (ck + 1) * 128],
                            anb[:, sj, ex : ex + 1],
                            start=(sj == 0),
                            stop=(sj == 6),
                        )
                nc.scalar.copy(ctxT[:, :, ex], c_ps)

            # ---- MLP head ----
            m_ps = pg.tile([BL, C], FDT, tag="g")
            for k in range(8):
                nc.tensor.matmul(
                    m_ps,
                    hT[:, k, :] if k < 4 else ctxT[:, k - 4, :],
                    w1_sb[:, k, :],
                    start=(k == 0),
                    stop=False,
                )
            nc.tensor.matmul(m_ps, ones8, b1_sb, start=False, stop=True)
            hid = work.tile([BL, C], FDT, tag="hid")
            nc.scalar.activation(hid, m_ps, AF.Tanh)

            hidT = work.tile([128, 4, BL], BDT, tag="hidT")
            for ck in range(4):
                tp = pt.tile([128, BL], FDT, tag="t")
                nc.tensor.transpose(tp, hid[:, ck * 128 : (ck + 1) * 128], id8f)
                nc.vector.tensor_copy(hidT[:, ck, :], tp)

            l_ps = pt.tile([BL, V], FDT, tag="t")
            for k in range(4):
                nc.tensor.matmul(
                    l_ps, hidT[:, k, :], w2_sb[:, k, :], start=(k == 0), stop=False
                )
            nc.tensor.matmul(l_ps, ones8, b2_sb, start=False, stop=True)
            o_sb = opool.tile([BL, V], FDT)
            nc.scalar.copy(o_sb, l_ps)
            nc.sync.dma_start(out=out_d[t], in_=o_sb)

    return nc


def prep_core_inputs(core, tokens, key_enc, value_enc, out_lens, emb, W_ih, W_hh,
                     b_ih, b_hh, W1, b1, W2, b2, t_steps=T):
    """Build the per-core input map (host-side shard + layout prep)."""
    sl = slice(core * BL, (core + 1) * BL)
    ke = key_enc[sl]  # [BL, S, H]
    kt = np.zeros((BL, H, SP), np.float32)
    kt[:, :, :S] = ke.transpose(0, 2, 1)
    kts = kt.reshape(BL, 4, 128, SP).astype(BF16)

    vv = np.zeros((BL, SP, C), np.float32)
    vv[:, :S] = value_enc[sl]
    v = vv.reshape(BL, 7, 128, C).astype(BF16)

    xe = emb[tokens[sl, :t_steps]]  # [BL, t, E]
    xembt = (
        xe.transpose(1, 2, 0).reshape(t_steps, 2, 128, BL).transpose(0, 2, 1, 3)
    ).astype(BF16)

    # column layout: mask[p, sj, ex] = (sj*128 + p) < out_lens[ex]
    m01 = (np.arange(SP)[None, :] < out_lens[sl][:, None]).astype(np.float32)
    mask = m01.reshape(BL, 7, 128).transpose(2, 1, 0)  # [128, 7, BL]

    return {
        "kts": np.ascontiguousarray(kts),
        "v": np.ascontiguousarray(v),
        "xembt": np.ascontiguousarray(xembt),
        "mask": np.ascontiguousarray(mask),
    }


def prep_shared_inputs(W_ih, W_hh, b_ih, b_hh, W1, b1, W2, b2):
    wc = np.concatenate([W_ih, W_hh], axis=1)  # [2048, 1280]
    wt = wc.T.reshape(10, 128, 4 * H).astype(BF16)
    return {
        "wt": np.ascontiguousarray(wt),
        "w1t": np.ascontiguousarray(W1.T.reshape(8, 128, C).astype(BF16)),
        "w2t": np.ascontiguousarray(W2.T.reshape(4, 128, V).astype(BF16)),
        "brow": np.ascontiguousarray((b_ih + b_hh)[None, :].astype(np.float32)),
        "b1row": np.ascontiguousarray(b1[None, :].astype(np.float32)),
        "b2row": np.ascontiguousarray(b2[None, :].astype(np.float32)),
    }


_CACHE = {}


def run(t_steps=T, trace=False, **inputs):
    from concourse.bass_utils import run_bass_kernel_spmd

    args = {k: np.asarray(v) for k, v in inputs.items()}
    tokens = args["tokens"].astype(np.int64)
    shared = prep_shared_inputs(
        args["W_ih"], args["W_hh"], args["b_ih"], args["b_hh"],
        args["W1"], args["b1"], args["W2"], args["b2"],
    )
    in_maps = []
    for core in range(NCORES):
        m = prep_core_inputs(
            core, tokens, args["key_enc"], args["value_enc"], args["out_lens"],
            args["emb"], None, None, None, None, None, None, None, None,
            t_steps=t_steps,
        )
        m.update(shared)
        in_maps.append(m)

    if t_steps not in _CACHE:
        nc_new = build_bass(t_steps)
        nc_new.finalize()  # runs bacc compile (event-semaphore wait split)
        _CACHE[t_steps] = nc_new
    nc = _CACHE[t_steps]

    res = run_bass_kernel_spmd(nc, in_maps, list(range(NCORES)), trace=trace)
    outs = [np.asarray(r["out"], np.float32) for r in res.results]
    full = np.concatenate(
        [o.transpose(1, 0, 2) for o in outs], axis=0
    )  # [B, t_steps, V]
    return full, res


def kernel(**inputs) -> np.ndarray:
    full, _ = run(t_steps=T, trace=False, **inputs)
    return full


def warm_timing(t_steps=T, n_iters=3, **inputs):
    """Time warm NEFF executions (device-resident inputs) as an HW-time proxy.

    Returns (best_seconds, out_full). Replicates run_bass_via_pjrt's
    shard_map path but keeps the jitted callable + device inputs so the
    repeat runs measure device execution (plus small dispatch overhead).
    """
    import time

    import jax
    from jax.sharding import Mesh, PartitionSpec
    from jax.experimental.shard_map import shard_map

    from concourse import bass2jax
    from concourse import mybir as _mybir
    from concourse.bass2jax import _bass_exec_p, install_neuronx_cc_hook

    install_neuronx_cc_hook()
    args = {k: np.asarray(v) for k, v in inputs.items()}
    tokens = args["tokens"].astype(np.int64)
    shared = prep_shared_inputs(
        args["W_ih"], args["W_hh"], args["b_ih"], args["b_hh"],
        args["W1"], args["b1"], args["W2"], args["b2"],
    )
    in_maps = []
    for core in range(NCORES):
        m = prep_core_inputs(
            core, tokens, args["key_enc"], args["value_enc"], args["out_lens"],
            args["emb"], None, None, None, None, None, None, None, None,
            t_steps=t_steps,
        )
        m.update(shared)
        in_maps.append(m)

    if t_steps not in _CACHE:
        nc_new = build_bass(t_steps)
        nc_new.finalize()
        _CACHE[t_steps] = nc_new
    nc = _CACHE[t_steps]

    partition_name = nc.partition_id_tensor.name if nc.partition_id_tensor else None
    in_names, out_names, out_avals, zero_outs = [], [], [], []
    for alloc in nc.m.functions[0].allocations:
        if not isinstance(alloc, _mybir.MemoryLocationSet):
            continue
        name = alloc.memorylocations[0].name
        if alloc.kind == "ExternalInput":
            if name != partition_name:
                in_names.append(name)
        elif alloc.kind == "ExternalOutput":
            out_names.append(name)
            shape = tuple(alloc.tensor_shape)
            dtype = _mybir.dt.np(alloc.dtype)
            out_avals.append(jax.core.ShapedArray(shape, dtype))
            zero_outs.append(np.zeros(shape, dtype))
    n_params = len(in_names)
    n_outs = len(out_avals)
    in_names.extend(out_names)
    if partition_name:
        in_names.append(partition_name)

    def _body(*a):
        operands = list(a)
        if partition_name:
            operands.append(bass2jax.partition_id_tensor())
        return tuple(
            _bass_exec_p.bind(
                *operands,
                out_avals=tuple(out_avals),
                in_names=tuple(in_names),
                out_names=tuple(out_names),
                lowering_input_output_aliases=(),
                sim_require_finite=True,
                sim_require_nnan=True,
                nc=nc,
            )
        )

    devices = jax.devices()[:NCORES]
    mesh = Mesh(np.asarray(devices), ("core",))
    sharded = jax.jit(
        shard_map(
            _body,
            mesh=mesh,
            in_specs=(PartitionSpec("core"),) * (n_params + n_outs),
            out_specs=(PartitionSpec("core"),) * len(out_names),
            check_rep=False,
        ),
        keep_unused=True,
    )
    per_core = [[np.asarray(m[nm]) for nm in in_names[:n_params]] for m in in_maps]
    concat_in = [
        jax.device_put(np.concatenate([per_core[c][i] for c in range(NCORES)], axis=0))
        for i in range(n_params)
    ]
    concat_zeros = [
        jax.device_put(np.zeros((NCORES * z.shape[0], *z.shape[1:]), z.dtype))
        for z in zero_outs
    ]
    outs = sharded(*concat_in, *concat_zeros)
    jax.block_until_ready(outs)
    best = None
    for _ in range(n_iters):
        t0 = time.time()
        outs = sharded(*concat_in, *concat_zeros)
        jax.block_until_ready(outs)
        dt = time.time() - t0
        best = dt if best is None else min(best, dt)

    oarr = np.asarray(outs[out_names.index("out")]).reshape(
        NCORES, t_steps, BL, V
    )
    full = np.concatenate([oarr[c].transpose(1, 0, 2) for c in range(NCORES)], axis=0)
    return best, full



# revision 4
# speedup vs baseline: 1.8259x; 1.8259x over previous
"""Attention-LSTM decoder (nn_Decoder) Trainium2 Bass kernel.

Sharding: data-parallel over batch B=64 -> 8 cores x 8 examples.
Per core, everything (K^T, V, weights) is SBUF-resident; the T=200 step
recurrence is fully unrolled.

Per-step dataflow (per core, BL=8 local examples):
  gates  : PE stream MMs  psum[8,512] x4 chunks  (lhsT = featT [128,8] bf16,
           rhs = W^T [128,512] bf16, + f32 bias MM via ones-row)
  lstm   : ACT sigmoid/tanh from psum, DVE muls/adds, f32 states
  hT     : PE transpose [8,128]->[128,8] x4, copy to bf16
  energy : PE stream MMs out [1,448] x2 halves per ex (lhsT = hT col [128,1],
           rhs = K^T tile [128,448] bf16), exp via ACT (scale=1/sqrt(128))
  softmax: no max-subtraction (energies are small); fused mask-mult + row-sum
           (DVE tensor_tensor_reduce), reciprocal, per-row normalize to bf16
  attnT  : PE transpose [8,128]->[128,8] x7, copy bf16
  ctx    : PE weights-path MMs out [128c,1] accumulate over 7 s-chunks
           (lhsT = V tile [128s,128c] bf16 FWL, rhs = attnT col [128,1])
  mlp    : PE stream MMs [8,512] + tanh, transpose, logits [8,34], DMA out
"""

import math
import os
import sys

import numpy as np

sys.path.insert(0, "/opt/trn_rl_repo")

import ml_dtypes  # noqa: E402

import concourse.bass as bass  # noqa: E402
import concourse.bacc as bacc  # noqa: E402
import concourse.tile as tile  # noqa: E402
from concourse import mybir  # noqa: E402
from concourse.masks import make_identity  # noqa: E402

BF16 = ml_dtypes.bfloat16

V, E, H, C = 34, 256, 512, 512
B, T, S = 64, 200, 800
SP = 896  # padded S (7 * 128)
NCORES = 8
BL = B // NCORES  # 8 examples per core
SCALE = 1.0 / math.sqrt(128.0)
FDT = mybir.dt.float32
BDT = mybir.dt.bfloat16


def build_bass(t_steps: int = T) -> bass.Bass:
    nc = bacc.Bacc()

    kts_d = nc.dram_tensor("kts", [BL, 4, 128, SP], BDT, kind="ExternalInput")
    v_d = nc.dram_tensor("v", [BL, 7, 128, C], BDT, kind="ExternalInput")
    w_d = nc.dram_tensor("wt", [10, 128, 4 * H], BDT, kind="ExternalInput")
    w1_d = nc.dram_tensor("w1t", [8, 128, C], BDT, kind="ExternalInput")
    w2_d = nc.dram_tensor("w2t", [4, 128, V], BDT, kind="ExternalInput")
    xemb_d = nc.dram_tensor("xembt", [t_steps, 128, 2, BL], BDT, kind="ExternalInput")
    mask_d = nc.dram_tensor("mask", [128, 7, BL], FDT, kind="ExternalInput")
    brow_d = nc.dram_tensor("brow", [1, 4 * H], FDT, kind="ExternalInput")
    b1_d = nc.dram_tensor("b1row", [1, C], FDT, kind="ExternalInput")
    b2_d = nc.dram_tensor("b2row", [1, V], FDT, kind="ExternalInput")
    out_d = nc.dram_tensor("out", [t_steps, BL, V], FDT, kind="ExternalOutput")

    from contextlib import ExitStack

    with tile.TileContext(nc) as tc, ExitStack() as es:
        consts = es.enter_context(tc.tile_pool(name="consts", bufs=1))
        state = es.enter_context(tc.tile_pool(name="state", bufs=1))
        work = es.enter_context(tc.tile_pool(name="work", bufs=1))
        xpool = es.enter_context(tc.tile_pool(name="xpool", bufs=4))
        opool = es.enter_context(tc.tile_pool(name="opool", bufs=4))
        pg = es.enter_context(tc.tile_pool(name="pg", bufs=2, space="PSUM"))
        pe_ = es.enter_context(tc.tile_pool(name="pe", bufs=2, space="PSUM"))
        pc = es.enter_context(tc.tile_pool(name="pc", bufs=2, space="PSUM"))
        pt = es.enter_context(tc.tile_pool(name="pt", bufs=2, space="PSUM"))

        # ---- resident tensors ----
        kts_sb = consts.tile([128, BL, 4, SP], BDT)
        for ex in range(BL):
            for hk in range(4):
                nc.sync.dma_start(out=kts_sb[:, ex, hk, :], in_=kts_d[ex, hk])
        v_sb = consts.tile([128, BL, 7, C], BDT)
        for ex in range(BL):
            for sj in range(7):
                nc.sync.dma_start(out=v_sb[:, ex, sj, :], in_=v_d[ex, sj])
        w_sb = consts.tile([128, 10, 4 * H], BDT)
        for k in range(10):
            nc.sync.dma_start(out=w_sb[:, k, :], in_=w_d[k])
        w1_sb = consts.tile([128, 8, C], BDT)
        for k in range(8):
            nc.sync.dma_start(out=w1_sb[:, k, :], in_=w1_d[k])
        w2_sb = consts.tile([128, 4, V], BDT)
        for k in range(4):
            nc.sync.dma_start(out=w2_sb[:, k, :], in_=w2_d[k])
        maskc_sb = consts.tile([128, 7, BL], FDT)
        nc.sync.dma_start(out=maskc_sb, in_=mask_d[:, :, :])
        brow_sb = consts.tile([1, 4 * H], FDT)
        nc.sync.dma_start(out=brow_sb, in_=brow_d[:, :])
        b1_sb = consts.tile([1, C], FDT)
        nc.sync.dma_start(out=b1_sb, in_=b1_d[:, :])
        b2_sb = consts.tile([1, V], FDT)
        nc.sync.dma_start(out=b2_sb, in_=b2_d[:, :])
        ones8 = consts.tile([1, BL], FDT)
        nc.vector.memset(ones8, 1.0)
        ones128c = consts.tile([128, 1], FDT)
        nc.vector.memset(ones128c, 1.0)
        ones128r = consts.tile([1, 128], FDT)
        nc.vector.memset(ones128r, 1.0)
        id8f = consts.tile([BL, BL], FDT)
        make_identity(nc, id8f)

        # ---- recurrent state ----
        hT = state.tile([128, 4, BL], BDT)  # h^T, feature-on-partition
        ctxT = state.tile([128, 4, BL], BDT)  # ctx^T
        cst = state.tile([BL, H], FDT)  # cell state, row layout
        nc.vector.memset(hT, 0.0)
        nc.vector.memset(ctxT, 0.0)
        nc.vector.memset(cst, 0.0)

        AF = mybir.ActivationFunctionType
        OP = mybir.AluOpType

        for t in range(t_steps):
            xT = xpool.tile([128, 2, BL], BDT)
            nc.sync.dma_start(out=xT, in_=xemb_d[t])

            def feat(k):
                if k < 2:
                    return xT[:, k, :]
                if k < 6:
                    return ctxT[:, k - 2, :]
                return hT[:, k - 6, :]

            # ---- LSTM gates: psum [8,512] per gate ----
            gact = []  # sbuf tiles: sig_i, sig_f, tanh_g, sig_o
            for n in range(4):
                g_ps = pg.tile([BL, 512], FDT, tag="g")
                for k in range(10):
                    nc.tensor.matmul(
                        g_ps,
                        feat(k),
                        w_sb[:, k, n * 512 : (n + 1) * 512],
                        start=(k == 0),
                        stop=False,
                    )
                nc.tensor.matmul(
                    g_ps,
                    ones8,
                    brow_sb[:, n * 512 : (n + 1) * 512],
                    start=False,
                    stop=True,
                )
                ga = work.tile([BL, 512], FDT, tag=f"ga{n}")
                nc.scalar.activation(ga, g_ps, AF.Tanh if n == 2 else AF.Sigmoid)
                gact.append(ga)

            # tensor_tensor_reduce instead of tensor_tensor: the plain TT ISA
            # struct only has one sync-wait slot (codegen rejects 2+ waits);
            # TTR lowers to an ISA struct with enough slots. accum is unused.
            def tt(out, a, b, op):
                nc.vector.scalar_tensor_tensor(
                    out=out, in0=a, scalar=1.0, in1=b, op0=OP.mult, op1=op
                )

            # keep every 2-tensor DVE op's inputs produced by ACT only (the
            # DVE 2-src ISA structs have a single sync-wait slot): bounce the
            # cell state through an ACT copy each step.
            c_act = work.tile([BL, H], FDT, tag="c_act")
            nc.scalar.copy(c_act, cst)
            t_ig = work.tile([BL, H], FDT, tag="t_ig")
            tt(t_ig, gact[0], gact[2], OP.mult)
            t_fc = work.tile([BL, H], FDT, tag="t_fc")
            tt(t_fc, gact[1], c_act, OP.mult)
            c_new = work.tile([BL, H], FDT, tag=f"c{t % 2}")
            tt(c_new, t_ig, t_fc, OP.add)
            cst = c_new
            tnc = work.tile([BL, H], FDT, tag="tnc")
            nc.scalar.activation(tnc, cst, AF.Tanh)
            h_sb = work.tile([BL, H], FDT, tag="h_sb")
            tt(h_sb, gact[3], tnc, OP.mult)

            # ---- h^T (bf16) via PE transpose ----
            for ck in range(4):
                tp = pt.tile([128, BL], FDT, tag="t")
                nc.tensor.transpose(tp, h_sb[:, ck * 128 : (ck + 1) * 128], id8f)
                nc.vector.tensor_copy(hT[:, ck, :], tp)

            # ---- attention: energy column-form [128s, 7] per example ----
            # e[s] = sum_h K^T[h, s] * h[h]; exp(scale*e); mask-mult with
            # fused per-partition partial sums; partition-sum via ones-MM.
            expm = work.tile([128, 7, BL], FDT, tag="expm")
            amb = work.tile([128, 7, BL], BDT, tag="amb")
            partials = work.tile([128, BL], FDT, tag="partials")
            for ex in range(BL):
                e_ps = pe_.tile([128, 7], FDT, tag="e")
                for sj in range(7):
                    for hk in range(4):
                        nc.tensor.matmul(
                            e_ps[:, sj : sj + 1],
                            kts_sb[:, ex, hk, sj * 128 : (sj + 1) * 128],
                            hT[:, hk, ex : ex + 1],
                            start=(hk == 0),
                            stop=(hk == 3),
                        )
                nc.scalar.activation(expm[:, :, ex], e_ps, AF.Exp, scale=SCALE)
                nc.vector.scalar_tensor_tensor(
                    out=amb[:, :, ex],
                    in0=expm[:, :, ex],
                    scalar=1.0,
                    in1=maskc_sb[:, :, ex],
                    op0=OP.mult,
                    op1=OP.mult,
                    accum_out=partials[:, ex : ex + 1],
                )

            es_ps = pt.tile([1, BL], FDT, tag="t")
            nc.tensor.matmul(es_ps, ones128c, partials, start=True, stop=True)
            rinv8 = work.tile([1, BL], FDT, tag="rinv8")
            nc.vector.reciprocal(rinv8, es_ps)
            rb_ps = pt.tile([128, BL], FDT, tag="t")
            nc.tensor.matmul(rb_ps, ones128r, rinv8, start=True, stop=True)
            rb_sb = work.tile([128, BL], FDT, tag="rb_sb")
            nc.scalar.copy(rb_sb, rb_ps)
            anb = work.tile([128, 7, BL], BDT, tag="anb")
            for ex in range(BL):
                nc.vector.tensor_scalar_mul(
                    anb[:, :, ex], amb[:, :, ex], rb_sb[:, ex : ex + 1]
                )

            # ---- context: ctx^T[128c, ex] accumulated over 7 s-chunks ----
            for ex in range(BL):
                c_ps = pc.tile([128, 4], FDT, tag="c")
                for ck in range(4):
                    for sj in range(7):
                        nc.tensor.matmul(
                            c_ps[:, ck : ck + 1],
                            v_sb[:, ex, sj, ck * 128 : (ck + 1) * 128],
                            anb[:, sj, ex : ex + 1],
                            start=(sj == 0),
                            stop=(sj == 6),
                        )
                nc.scalar.copy(ctxT[:, :, ex], c_ps)

            # ---- MLP head ----
            m_ps = pg.tile([BL, C], FDT, tag="g")
            for k in range(8):
                nc.tensor.matmul(
                    m_ps,
                    hT[:, k, :] if k < 4 else ctxT[:, k - 4, :],
                    w1_sb[:, k, :],
                    start=(k == 0),
                    stop=False,
                )
            nc.tensor.matmul(m_ps, ones8, b1_sb, start=False, stop=True)
            hid = work.tile([BL, C], FDT, tag="hid")
            nc.scalar.activation(hid, m_ps, AF.Tanh)

            hidT = work.tile([128, 4, BL], BDT, tag="hidT")
            for ck in range(4):
                tp = pt.tile([128, BL], FDT, tag="t")
                nc.tensor.transpose(tp, hid[:, ck * 128 : (ck + 1) * 128], id8f)
                nc.vector.tensor_copy(hidT[:, ck, :], tp)

            l_ps = pt.tile([BL, V], FDT, tag="t")
            for k in range(4):
                nc.tensor.matmul(
                    l_ps, hidT[:, k, :], w2_sb[:, k, :], start=(k == 0), stop=False
                )
            nc.tensor.matmul(l_ps, ones8, b2_sb, start=False, stop=True)
            o_sb = opool.tile([BL, V], FDT)
            nc.scalar.copy(o_sb, l_ps)
            nc.sync.dma_start(out=out_d[t], in_=o_sb)

    return nc


def prep_core_inputs(core, tokens, key_enc, value_enc, out_lens, emb, W_ih, W_hh,
                     b_ih, b_hh, W1, b1, W2, b2, t_steps=T):
    """Build the per-core input map (host-side shard + layout prep)."""
    sl = slice(core * BL, (core + 1) * BL)
    ke = key_enc[sl]  # [BL, S, H]
    kt = np.zeros((BL, H, SP), np.float32)
    kt[:, :, :S] = ke.transpose(0, 2, 1)
    kts = kt.reshape(BL, 4, 128, SP).astype(BF16)

    vv = np.zeros((BL, SP, C), np.float32)
    vv[:, :S] = value_enc[sl]
    v = vv.reshape(BL, 7, 128, C).astype(BF16)

    xe = emb[tokens[sl, :t_steps]]  # [BL, t, E]
    xembt = (
        xe.transpose(1, 2, 0).reshape(t_steps, 2, 128, BL).transpose(0, 2, 1, 3)
    ).astype(BF16)

    # column layout: mask[p, sj, ex] = (sj*128 + p) < out_lens[ex]
    m01 = (np.arange(SP)[None, :] < out_lens[sl][:, None]).astype(np.float32)
    mask = m01.reshape(BL, 7, 128).transpose(2, 1, 0)  # [128, 7, BL]

    return {
        "kts": np.ascontiguousarray(kts),
        "v": np.ascontiguousarray(v),
        "xembt": np.ascontiguousarray(xembt),
        "mask": np.ascontiguousarray(mask),
    }


def prep_shared_inputs(W_ih, W_hh, b_ih, b_hh, W1, b1, W2, b2):
    wc = np.concatenate([W_ih, W_hh], axis=1)  # [2048, 1280]
    wt = wc.T.reshape(10, 128, 4 * H).astype(BF16)
    return {
        "wt": np.ascontiguousarray(wt),
        "w1t": np.ascontiguousarray(W1.T.reshape(8, 128, C).astype(BF16)),
        "w2t": np.ascontiguousarray(W2.T.reshape(4, 128, V).astype(BF16)),
        "brow": np.ascontiguousarray((b_ih + b_hh)[None, :].astype(np.float32)),
        "b1row": np.ascontiguousarray(b1[None, :].astype(np.float32)),
        "b2row": np.ascontiguousarray(b2[None, :].astype(np.float32)),
    }


_CACHE = {}


def run(t_steps=T, trace=False, **inputs):
    from concourse.bass_utils import run_bass_kernel_spmd

    args = {k: np.asarray(v) for k, v in inputs.items()}
    tokens = args["tokens"].astype(np.int64)
    shared = prep_shared_inputs(
        args["W_ih"], args["W_hh"], args["b_ih"], args["b_hh"],
        args["W1"], args["b1"], args["W2"], args["b2"],
    )
    in_maps = []
    for core in range(NCORES):
        m = prep_core_inputs(
            core, tokens, args["key_enc"], args["value_enc"], args["out_lens"],
            args["emb"], None, None, None, None, None, None, None, None,
            t_steps=t_steps,
        )
        m.update(shared)
        in_maps.append(m)

    if t_steps not in _CACHE:
        nc_new = build_bass(t_steps)
        nc_new.finalize()  # runs bacc compile (event-semaphore wait split)
        _CACHE[t_steps] = nc_new
    nc = _CACHE[t_steps]

    res = run_bass_kernel_spmd(nc, in_maps, list(range(NCORES)), trace=trace)
    outs = [np.asarray(r["out"], np.float32) for r in res.results]
    full = np.concatenate(
        [o.transpose(1, 0, 2) for o in outs], axis=0
    )  # [B, t_steps, V]
    return full, res


def kernel(**inputs) -> np.ndarray:
    full, _ = run(t_steps=T, trace=False, **inputs)
    return full


def warm_timing(t_steps=T, n_iters=3, **inputs):
    """Time warm NEFF executions (device-resident inputs) as an HW-time proxy.

    Returns (best_seconds, out_full). Replicates run_bass_via_pjrt's
    shard_map path but keeps the jitted callable + device inputs so the
    repeat runs measure device execution (plus small dispatch overhead).
    """
    import time

    import jax
    from jax.sharding import Mesh, NamedSharding, PartitionSpec
    from jax.experimental.shard_map import shard_map

    from concourse import bass2jax
    from concourse import mybir as _mybir
    from concourse.bass2jax import _bass_exec_p, install_neuronx_cc_hook

    install_neuronx_cc_hook()
    args = {k: np.asarray(v) for k, v in inputs.items()}
    tokens = args["tokens"].astype(np.int64)
    shared = prep_shared_inputs(
        args["W_ih"], args["W_hh"], args["b_ih"], args["b_hh"],
        args["W1"], args["b1"], args["W2"], args["b2"],
    )
    in_maps = []
    for core in range(NCORES):
        m = prep_core_inputs(
            core, tokens, args["key_enc"], args["value_enc"], args["out_lens"],
            args["emb"], None, None, None, None, None, None, None, None,
            t_steps=t_steps,
        )
        m.update(shared)
        in_maps.append(m)

    if t_steps not in _CACHE:
        nc_new = build_bass(t_steps)
        nc_new.finalize()
        _CACHE[t_steps] = nc_new
    nc = _CACHE[t_steps]

    partition_name = nc.partition_id_tensor.name if nc.partition_id_tensor else None
    in_names, out_names, out_avals, zero_outs = [], [], [], []
    for alloc in nc.m.functions[0].allocations:
        if not isinstance(alloc, _mybir.MemoryLocationSet):
            continue
        name = alloc.memorylocations[0].name
        if alloc.kind == "ExternalInput":
            if name != partition_name:
                in_names.append(name)
        elif alloc.kind == "ExternalOutput":
            out_names.append(name)
            shape = tuple(alloc.tensor_shape)
            dtype = _mybir.dt.np(alloc.dtype)
            out_avals.append(jax.core.ShapedArray(shape, dtype))
            zero_outs.append(np.zeros(shape, dtype))
    n_params = len(in_names)
    n_outs = len(out_avals)
    in_names.extend(out_names)
    if partition_name:
        in_names.append(partition_name)

    def _body(*a):
        operands = list(a)
        if partition_name:
            operands.append(bass2jax.partition_id_tensor())
        return tuple(
            _bass_exec_p.bind(
                *operands,
                out_avals=tuple(out_avals),
                in_names=tuple(in_names),
                out_names=tuple(out_names),
                lowering_input_output_aliases=(),
                sim_require_finite=True,
                sim_require_nnan=True,
                nc=nc,
            )
        )

    devices = jax.devices()[:NCORES]
    mesh = Mesh(np.asarray(devices), ("core",))
    sharded = jax.jit(
        shard_map(
            _body,
            mesh=mesh,
            in_specs=(PartitionSpec("core"),) * (n_params + n_outs),
            out_specs=(PartitionSpec("core"),) * len(out_names),
            check_rep=False,
        ),
        keep_unused=True,
    )
    per_core = [[np.asarray(m[nm]) for nm in in_names[:n_params]] for m in in_maps]
    shard = NamedSharding(mesh, PartitionSpec("core"))
    concat_in = [
        jax.device_put(
            np.concatenate([per_core[c][i] for c in range(NCORES)], axis=0), shard
        )
        for i in range(n_params)
    ]
    concat_zeros = [
        jax.device_put(np.zeros((NCORES * z.shape[0], *z.shape[1:]), z.dtype), shard)
        for z in zero_outs
    ]
    outs = sharded(*concat_in, *concat_zeros)
    jax.block_until_ready(outs)
    best = None
    times = []
    for _ in range(n_iters):
        t0 = time.time()
        outs = sharded(*concat_in, *concat_zeros)
        jax.block_until_ready(outs)
        dt = time.time() - t0
        times.append(dt)
        best = dt if best is None else min(best, dt)
    print(f"warm iters (s): {[f'{x:.4f}' for x in times]}")

    oarr = np.asarray(outs[out_names.index("out")]).reshape(
        NCORES, t_steps, BL, V
    )
    full = np.concatenate([oarr[c].transpose(1, 0, 2) for c in range(NCORES)], axis=0)
    return best, full



# revision 27
# speedup vs baseline: 2.2946x; 1.2567x over previous
"""Attention-LSTM decoder (nn_Decoder) Trainium2 Bass kernel — v2.

Sharding: data-parallel over batch B=64 -> 8 cores x 8 examples, with
examples assigned to (core, slot) by descending attention length so the
shared SPMD program can give each slot its own (minimal) number of
128-wide S chunks (`slot_cnts`), skipping fully-masked K/V chunks.

Layouts are column-major (feature-on-partition) for all small per-step
tensors so ACT/DVE ops are short in the free dim; LSTM gates use four
base-partition offsets (0/32/64/96) of ONE PSUM bank, which bass lowers
to 4-way col-tiled (concurrent) matmuls.

Sigmoid is computed as tanh(x/2) (sig(x) = (tanh(x/2)+1)/2) with the
doubling folded into W_hh / W1 host-side and into the exp scale, so all
activation functions per step {Tanh, Exp, Copy} live in one ACT table
set (`exp_and_others`) — zero activation-table reloads in steady state.

Per-step emission order (step t):
  energy_t (PE col-form MMs, packed psum) -> exp_t (1 ACT op)
  -> mask-mult+partials (8 DVE STT) -> [xT_{t+1} DMA]
  -> ctx_t MMs (unnormalized, interleaved with gates_{t+1} x/h chunks)
  -> rowsum/recip/bcast (PE/DVE, concurrent with ctx MMs)
  -> ctxT_t = ctx_psum * rinv (ACT copy + 8 DVE TSP)
  -> MLP_t (m_ps, tanh, hidT transposes, logits, out DMA)
  -> gates_{t+1} ctx chunks + bias -> LSTM_{t+1} -> h2T_{t+1}
"""

import math
import os
import sys

import numpy as np

sys.path.insert(0, "/opt/trn_rl_repo")

import ml_dtypes  # noqa: E402

import concourse.bass as bass  # noqa: E402
import concourse.bacc as bacc  # noqa: E402
import concourse.tile as tile  # noqa: E402
from concourse import mybir  # noqa: E402
from concourse.masks import make_identity  # noqa: E402

BF16 = ml_dtypes.bfloat16

V, E, H, C = 34, 256, 512, 512
B, T, S = 64, 200, 800
NCORES = 8
BL = B // NCORES  # 8 example slots per core
SCALE = 1.0 / math.sqrt(128.0)
FDT = mybir.dt.float32
BDT = mybir.dt.bfloat16


def build_bass(t_steps: int, slot_cnts: tuple) -> bass.Bass:
    assert len(slot_cnts) == BL
    offs = [0]
    for c in slot_cnts:
        offs.append(offs[-1] + c)
    tot = offs[-1]

    nc = bacc.Bacc()

    kts_d = nc.dram_tensor("kts", [4, 128, tot * 128], BDT, kind="ExternalInput")
    v_d = nc.dram_tensor("v", [128, tot, C], BDT, kind="ExternalInput")
    w_d = nc.dram_tensor("wt", [10, 128, 4 * H], BDT, kind="ExternalInput")
    w1_d = nc.dram_tensor("w1t", [8, 128, C], BDT, kind="ExternalInput")
    w2_d = nc.dram_tensor("w2t", [4, 128, V], BDT, kind="ExternalInput")
    xemb_d = nc.dram_tensor("xembt", [128, t_steps, 2, BL], BDT, kind="ExternalInput")
    mask_d = nc.dram_tensor("mask", [128, tot], FDT, kind="ExternalInput")
    brow_d = nc.dram_tensor("brow", [1, 4 * H], FDT, kind="ExternalInput")
    b1_d = nc.dram_tensor("b1row", [1, C], FDT, kind="ExternalInput")
    b2_d = nc.dram_tensor("b2row", [1, V], FDT, kind="ExternalInput")
    out_d = nc.dram_tensor("out", [BL, t_steps, V], FDT, kind="ExternalOutput")

    from contextlib import ExitStack

    AF = mybir.ActivationFunctionType
    OP = mybir.AluOpType

    with tile.TileContext(nc) as tc, ExitStack() as es:
        consts = es.enter_context(tc.tile_pool(name="consts", bufs=1))
        state = es.enter_context(tc.tile_pool(name="state", bufs=1))
        work = es.enter_context(tc.tile_pool(name="work", bufs=1))
        pg = es.enter_context(tc.tile_pool(name="pg", bufs=1, space="PSUM"))
        pa = es.enter_context(tc.tile_pool(name="pa", bufs=2, space="PSUM"))
        pt = es.enter_context(tc.tile_pool(name="pt", bufs=2, space="PSUM"))

        # ---- resident tensors ----
        kts_sb = consts.tile([128, 4, tot * 128], BDT)
        for hk in range(4):
            nc.sync.dma_start(out=kts_sb[:, hk, :], in_=kts_d[hk])
        v_sb = consts.tile([128, tot, C], BDT)
        nc.sync.dma_start(out=v_sb, in_=v_d[:, :, :])
        w_sb = consts.tile([128, 10, 4 * H], BDT)
        for k in range(10):
            nc.sync.dma_start(out=w_sb[:, k, :], in_=w_d[k])
        w1_sb = consts.tile([128, 8, C], BDT)
        for k in range(8):
            nc.sync.dma_start(out=w1_sb[:, k, :], in_=w1_d[k])
        w2_sb = consts.tile([128, 4, V], BDT)
        for k in range(4):
            nc.sync.dma_start(out=w2_sb[:, k, :], in_=w2_d[k])
        maskp_sb = consts.tile([128, tot], FDT)
        nc.sync.dma_start(out=maskp_sb, in_=mask_d[:, :])
        brow_sb = consts.tile([1, 4 * H], FDT)
        nc.sync.dma_start(out=brow_sb, in_=brow_d[:, :])
        b1_sb = consts.tile([1, C], FDT)
        nc.sync.dma_start(out=b1_sb, in_=b1_d[:, :])
        b2_sb = consts.tile([1, V], FDT)
        nc.sync.dma_start(out=b2_sb, in_=b2_d[:, :])
        # all x embeddings resident: [128, t, 2, BL] (t*32 bf16 cols/partition)
        xemb_sb = consts.tile([128, t_steps, 2, BL], BDT)
        nc.sync.dma_start(out=xemb_sb, in_=xemb_d[:, :, :, :])
        # output staging: [BL, t, V] fp32, one DMA at the end
        outbuf = consts.tile([BL, t_steps, V], FDT)
        ones8 = consts.tile([1, BL], FDT)
        nc.vector.memset(ones8, 1.0)
        ones128c = consts.tile([128, 1], FDT)
        nc.vector.memset(ones128c, 1.0)
        ones128r = consts.tile([1, 128], FDT)
        nc.vector.memset(ones128r, 1.0)
        id8f = consts.tile([BL, BL], FDT)
        make_identity(nc, id8f)
        # 8x8 identity replicated at each 32-row strip (transpose operands
        # must share a base partition with their identity)
        id_strips = consts.tile([128, BL], FDT)
        for n in range(4):
            make_identity(nc, id_strips[32 * n : 32 * n + BL, :])

        # ---- recurrent state (all column layout) ----
        h2T = state.tile([128, 4, BL], BDT)  # (2h)^T columns
        ctxT = state.tile([128, 4, BL], BDT)  # ctx^T columns
        c2 = state.tile([128, 4, BL], FDT)  # doubled cell state (2c)^T columns
        nc.vector.memset(h2T, 0.0)
        nc.vector.memset(ctxT, 0.0)
        nc.vector.memset(c2, 0.0)

        XH_CHUNKS = [0, 1, 6, 7, 8, 9]
        CTX_CHUNKS = [2, 3, 4, 5]

        def feat(k, xT):
            if k < 2:
                return xT[:, k, :]
            if k < 6:
                return ctxT[:, k - 2, :]
            return h2T[:, k - 6, :]

        def gate_mm(g_ps, n, k, xT, started):
            nc.tensor.matmul(
                g_ps[32 * n : 32 * n + BL, :],
                feat(k, xT),
                w_sb[:, k, n * 512 : (n + 1) * 512],
                start=not started,
                stop=False,
                tile_position=(0, 32 * n),
            )

        def gate_bias(g_ps, n):
            nc.tensor.matmul(
                g_ps[32 * n : 32 * n + BL, :],
                ones8,
                brow_sb[:, n * 512 : (n + 1) * 512],
                start=False,
                stop=True,
                tile_position=(0, 32 * n),
            )

        def emit_lstm(g_ps, t):
            """gate psums -> h2T, c2 (column layout, doubled-state tanh form).

            Host-side gate order is (i, f, o, g):
            i', f', o' = tanh(gate/2) = 2*sig(gate) - 1 ; tg = tanh(g).
            """
            gact_all = work.tile([128, 512], FDT, tag="gact")
            # one ACT over partition rows 0..104 covers i/f/o (rows between
            # the 8-row gate strips hold junk that is never read)
            nc.scalar.activation(
                gact_all[0 : 64 + BL, :], g_ps[0 : 64 + BL, :], AF.Tanh, scale=0.5
            )
            nc.scalar.activation(
                gact_all[96 : 96 + BL, :], g_ps[96 : 96 + BL, :], AF.Tanh, scale=1.0
            )
            # transpose all 16 (gate, chunk) strips into one psum tile
            gt_ps = pt.tile([128, 4, 4, BL], FDT, tag="t")
            for n in range(4):
                for ck in range(4):
                    nc.tensor.transpose(
                        gt_ps[:, n, ck, :],
                        gact_all[32 * n : 32 * n + BL, ck * 128 : (ck + 1) * 128],
                        id_strips[32 * n : 32 * n + BL, :],
                    )
            ip, fp, op_, tg = (gt_ps[:, n] for n in range(4))
            # c_act = c2/2 = c  (ACT bounce keeps DVE waits single-source)
            c_act = work.tile([128, 4, BL], FDT, tag="c_act")
            nc.scalar.activation(c_act, c2, AF.Copy, scale=0.5)
            t_ig = work.tile([128, 4, BL], FDT, tag="t_ig")
            nc.vector.scalar_tensor_tensor(
                out=t_ig, in0=ip, scalar=1.0, in1=tg, op0=OP.add, op1=OP.mult
            )
            t_fc = work.tile([128, 4, BL], FDT, tag="t_fc")
            nc.vector.scalar_tensor_tensor(
                out=t_fc, in0=fp, scalar=1.0, in1=c_act, op0=OP.add, op1=OP.mult
            )
            # c2_new = t_ig + t_fc = 2*c_new  (overwrites state tile)
            nc.vector.scalar_tensor_tensor(
                out=c2, in0=t_ig, scalar=1.0, in1=t_fc, op0=OP.mult, op1=OP.add
            )
            tnc = work.tile([128, 4, BL], FDT, tag="tnc")
            nc.scalar.activation(tnc, c2, AF.Tanh, scale=0.5)  # tanh(c)
            # h2T = (o' + 1) * tanh(c), written straight into the bf16 state
            nc.vector.scalar_tensor_tensor(
                out=h2T, in0=op_, scalar=1.0, in1=tnc, op0=OP.add, op1=OP.mult
            )

        for t in range(t_steps):
            if t == 0:
                xT0 = xemb_sb[:, 0]
                g_ps = pg.tile([128, 512], FDT, tag="g")
                # zero the full bank once so the junk rows between gate
                # strips read as finite values forever after
                nc.vector.memset(g_ps, 0.0)
                for n in range(4):
                    for j, k in enumerate(XH_CHUNKS + CTX_CHUNKS):
                        gate_mm(g_ps, n, k, xT0, started=(j > 0))
                    gate_bias(g_ps, n)
                emit_lstm(g_ps, 0)

            # ---- attention energies: packed column-form psum [128, tot] ----
            e_all = pa.tile([128, tot], FDT, tag="a")
            for ex in range(BL):
                for sj in range(slot_cnts[ex]):
                    col = offs[ex] + sj
                    for hk in range(4):
                        nc.tensor.matmul(
                            e_all[:, col : col + 1],
                            kts_sb[:, hk, col * 128 : (col + 1) * 128],
                            h2T[:, hk, ex : ex + 1],
                            start=(hk == 0),
                            stop=(hk == 3),
                        )
            expm = work.tile([128, tot], FDT, tag="expm")
            # e_psum holds 2e (h2 = 2h), so scale by SCALE/2
            nc.scalar.activation(expm, e_all, AF.Exp, scale=SCALE * 0.5)

            # masked exp + per-(partition,ex) partial sums
            amb = work.tile([128, tot], BDT, tag="amb")
            partials = work.tile([128, BL], FDT, tag="partials")
            for ex in range(BL):
                sl = slice(offs[ex], offs[ex + 1])
                nc.vector.scalar_tensor_tensor(
                    out=amb[:, sl],
                    in0=expm[:, sl],
                    scalar=1.0,
                    in1=maskp_sb[:, sl],
                    op0=OP.mult,
                    op1=OP.mult,
                    accum_out=partials[:, ex : ex + 1],
                )

            if t + 1 < t_steps:
                xT = xemb_sb[:, t + 1]

            # softmax denominator (runs concurrent with ctx MMs below)
            es_ps = pt.tile([1, BL], FDT, tag="t")
            nc.tensor.matmul(es_ps, ones128c, partials, start=True, stop=True)
            rinv8 = work.tile([1, BL], FDT, tag="rinv8")
            nc.vector.reciprocal(rinv8, es_ps)
            rb_ps = pt.tile([128, BL], FDT, tag="t")
            nc.tensor.matmul(rb_ps, ones128r, rinv8, start=True, stop=True)
            rb_sb = work.tile([128, BL], FDT, tag="rb_sb")
            nc.scalar.copy(rb_sb, rb_ps)

            # ---- ctx (unnormalized) + interleaved gates_{t+1} x/h chunks ----
            do_next = t + 1 < t_steps
            if do_next:
                g_ps = pg.tile([128, 512], FDT, tag="g")
                gq = [(n, k) for n in range(4) for k in XH_CHUNKS]
                gstarted = [False] * 4
            c_ps = pa.tile([128, BL, 4], FDT, tag="a")
            for ex in range(BL):
                for ck in range(4):
                    ccol = ex * 4 + ck
                    for j, sj in enumerate(range(slot_cnts[ex])):
                        nc.tensor.matmul(
                            c_ps[:, ex, ck : ck + 1],
                            v_sb[:, offs[ex] + sj, ck * 128 : (ck + 1) * 128],
                            amb[:, offs[ex] + sj : offs[ex] + sj + 1],
                            start=(j == 0),
                            stop=(j == slot_cnts[ex] - 1),
                        )
                if do_next:
                    for _ in range(3):
                        if gq:
                            n, k = gq.pop(0)
                            gate_mm(g_ps, n, k, xT, started=gstarted[n])
                            gstarted[n] = True

            # ctxT = ctx_psum * (1/rowsum)  — copy + per-ex scale
            ctmp = work.tile([128, BL, 4], FDT, tag="ctmp")
            nc.scalar.copy(ctmp, c_ps)
            for ex in range(BL):
                nc.vector.tensor_scalar_mul(
                    ctxT[:, :, ex], ctmp[:, ex, :], rb_sb[:, ex : ex + 1]
                )

            # ---- MLP head ----
            m_ps = pt.tile([BL, C], FDT, tag="t")
            for k in range(8):
                nc.tensor.matmul(
                    m_ps,
                    h2T[:, k, :] if k < 4 else ctxT[:, k - 4, :],
                    w1_sb[:, k, :],
                    start=(k == 0),
                    stop=False,
                )
            nc.tensor.matmul(m_ps, ones8, b1_sb, start=False, stop=True)
            hid = work.tile([BL, C], FDT, tag="hid")
            nc.scalar.activation(hid, m_ps, AF.Tanh)

            hidT = work.tile([128, 4, BL], BDT, tag="hidT")
            hT_ps = pt.tile([128, 4, BL], FDT, tag="t")
            for ck in range(4):
                nc.tensor.transpose(
                    hT_ps[:, ck, :], hid[:, ck * 128 : (ck + 1) * 128], id8f
                )
            nc.vector.tensor_copy(hidT, hT_ps)

            l_ps = pt.tile([BL, V], FDT, tag="t")
            for k in range(4):
                nc.tensor.matmul(
                    l_ps, hidT[:, k, :], w2_sb[:, k, :], start=(k == 0), stop=False
                )
            nc.tensor.matmul(l_ps, ones8, b2_sb, start=False, stop=True)
            nc.scalar.copy(outbuf[:, t, :], l_ps)

            # ---- finish gates_{t+1} (ctx chunks + bias) and LSTM_{t+1} ----
            if do_next:
                for n in range(4):
                    for k in CTX_CHUNKS:
                        gate_mm(g_ps, n, k, xT, started=gstarted[n])
                        gstarted[n] = True
                    gate_bias(g_ps, n)
                emit_lstm(g_ps, t + 1)

        nc.sync.dma_start(out=out_d[:, :, :], in_=outbuf)

    return nc


def assign_slots(out_lens):
    """Assign examples to (core, slot) by descending chunk count.

    Returns (perm, slot_cnts): perm[j*NCORES + c] = example index placed at
    core c, slot j; slot_cnts[j] = max chunk count within slot j (shared by
    the SPMD program).
    """
    lens = np.asarray(out_lens).clip(1, S)
    cnt = np.ceil(lens / 128.0).astype(int)
    perm = np.argsort(-cnt, kind="stable")
    slot_cnts = tuple(int(cnt[perm[j * NCORES]]) for j in range(BL))
    return perm, slot_cnts


def prep_core_inputs(core, perm, slot_cnts, tokens, key_enc, value_enc, out_lens,
                     emb, t_steps):
    offs = np.concatenate([[0], np.cumsum(slot_cnts)]).astype(int)
    tot = int(offs[-1])
    exs = [int(perm[j * NCORES + core]) for j in range(BL)]

    kts = np.zeros((4, 128, tot * 128), np.float32)
    vp = np.zeros((128, tot, C), np.float32)
    maskp = np.zeros((128, tot), np.float32)
    for j, ex in enumerate(exs):
        cj = slot_cnts[j]
        scols = cj * 128
        ke = key_enc[ex, : min(scols, S)]  # [s, H]
        kt = np.zeros((H, scols), np.float32)
        kt[:, : ke.shape[0]] = ke.T
        kts[:, :, offs[j] * 128 : offs[j] * 128 + scols] = kt.reshape(4, 128, scols)
        vv = np.zeros((scols, C), np.float32)
        vv[: ke.shape[0]] = value_enc[ex, : ke.shape[0]]
        vp[:, offs[j] : offs[j] + cj, :] = vv.reshape(cj, 128, C).transpose(1, 0, 2)
        m = (np.arange(scols) < out_lens[ex]).astype(np.float32)
        maskp[:, offs[j] : offs[j] + cj] = m.reshape(cj, 128).T

    xe = emb[tokens[exs, :t_steps]]  # [BL, t, E]
    xembt = (
        xe.transpose(1, 2, 0).reshape(t_steps, 2, 128, BL).transpose(2, 0, 1, 3)
    ).astype(BF16)

    return {
        "kts": np.ascontiguousarray(kts.astype(BF16)),
        "v": np.ascontiguousarray(vp.astype(BF16)),
        "xembt": np.ascontiguousarray(xembt),
        "mask": np.ascontiguousarray(maskp),
    }


def prep_shared_inputs(W_ih, W_hh, b_ih, b_hh, W1, b1, W2, b2):
    # fold the doubled-h representation into the h-consuming weights, and
    # reorder gates (i,f,g,o) -> (i,f,o,g) so i/f/o share one ACT strip
    reord = np.r_[0 : 2 * H, 3 * H : 4 * H, 2 * H : 3 * H]
    wc = np.concatenate([W_ih, 0.5 * np.asarray(W_hh)], axis=1)[reord]  # [2048, 1280]
    wt = wc.T.reshape(10, 128, 4 * H).astype(BF16)
    W1h = np.array(W1, np.float32)
    W1h[:, :C] *= 0.5
    return {
        "wt": np.ascontiguousarray(wt),
        "w1t": np.ascontiguousarray(W1h.T.reshape(8, 128, C).astype(BF16)),
        "w2t": np.ascontiguousarray(np.asarray(W2).T.reshape(4, 128, V).astype(BF16)),
        "brow": np.ascontiguousarray(
            (np.asarray(b_ih) + np.asarray(b_hh))[reord][None, :].astype(np.float32)
        ),
        "b1row": np.ascontiguousarray(np.asarray(b1)[None, :].astype(np.float32)),
        "b2row": np.ascontiguousarray(np.asarray(b2)[None, :].astype(np.float32)),
    }


_CACHE = {}


def _build_cached(t_steps, slot_cnts):
    key = (t_steps, slot_cnts)
    if key not in _CACHE:
        nc = build_bass(t_steps, slot_cnts)
        nc.finalize()
        _CACHE[key] = nc
    return _CACHE[key]


def _prep_all(t_steps, inputs):
    args = {k: np.asarray(v) for k, v in inputs.items()}
    tokens = args["tokens"].astype(np.int64)
    out_lens = args["out_lens"].astype(np.int64)
    perm, slot_cnts = assign_slots(out_lens)
    shared = prep_shared_inputs(
        args["W_ih"], args["W_hh"], args["b_ih"], args["b_hh"],
        args["W1"], args["b1"], args["W2"], args["b2"],
    )
    in_maps = []
    for core in range(NCORES):
        m = prep_core_inputs(
            core, perm, slot_cnts, tokens, args["key_enc"], args["value_enc"],
            out_lens, args["emb"], t_steps,
        )
        m.update(shared)
        in_maps.append(m)
    return in_maps, perm, slot_cnts


def _unpermute(outs, perm, t_steps):
    """outs: list of per-core 'out' arrays [BL, t, V] -> full [B, t, V]."""
    full = np.empty((B, t_steps, V), np.float32)
    for core in range(NCORES):
        o = np.asarray(outs[core], np.float32)  # [BL, t, V]
        for j in range(BL):
            full[perm[j * NCORES + core]] = o[j]
    return full


def run(t_steps=T, trace=False, **inputs):
    from concourse.bass_utils import run_bass_kernel_spmd

    in_maps, perm, slot_cnts = _prep_all(t_steps, inputs)
    nc = _build_cached(t_steps, slot_cnts)
    res = run_bass_kernel_spmd(nc, in_maps, list(range(NCORES)), trace=trace)
    full = _unpermute([r["out"] for r in res.results], perm, t_steps)
    return full, res


def kernel(**inputs) -> np.ndarray:
    full, _ = run(t_steps=T, trace=False, **inputs)
    return full


def warm_timing(t_steps=T, n_iters=5, **inputs):
    """Time warm NEFF executions (device-resident, properly sharded inputs).

    Returns (best_seconds, out_full)."""
    import time

    import jax
    from jax.sharding import Mesh, NamedSharding, PartitionSpec
    from jax.experimental.shard_map import shard_map

    from concourse import bass2jax
    from concourse import mybir as _mybir
    from concourse.bass2jax import _bass_exec_p, install_neuronx_cc_hook

    install_neuronx_cc_hook()
    in_maps, perm, slot_cnts = _prep_all(t_steps, inputs)
    nc = _build_cached(t_steps, slot_cnts)

    partition_name = nc.partition_id_tensor.name if nc.partition_id_tensor else None
    in_names, out_names, out_avals, zero_outs = [], [], [], []
    for alloc in nc.m.functions[0].allocations:
        if not isinstance(alloc, _mybir.MemoryLocationSet):
            continue
        name = alloc.memorylocations[0].name
        if alloc.kind == "ExternalInput":
            if name != partition_name:
                in_names.append(name)
        elif alloc.kind == "ExternalOutput":
            out_names.append(name)
            shape = tuple(alloc.tensor_shape)
            dtype = _mybir.dt.np(alloc.dtype)
            out_avals.append(jax.core.ShapedArray(shape, dtype))
            zero_outs.append(np.zeros(shape, dtype))
    n_params = len(in_names)
    n_outs = len(out_avals)
    in_names.extend(out_names)
    if partition_name:
        in_names.append(partition_name)

    def _body(*a):
        operands = list(a)
        if partition_name:
            operands.append(bass2jax.partition_id_tensor())
        return tuple(
            _bass_exec_p.bind(
                *operands,
                out_avals=tuple(out_avals),
                in_names=tuple(in_names),
                out_names=tuple(out_names),
                lowering_input_output_aliases=(),
                sim_require_finite=True,
                sim_require_nnan=True,
                nc=nc,
            )
        )

    devices = jax.devices()[:NCORES]
    mesh = Mesh(np.asarray(devices), ("core",))
    sharded = jax.jit(
        shard_map(
            _body,
            mesh=mesh,
            in_specs=(PartitionSpec("core"),) * (n_params + n_outs),
            out_specs=(PartitionSpec("core"),) * len(out_names),
            check_rep=False,
        ),
        keep_unused=True,
    )
    per_core = [[np.asarray(m[nm]) for nm in in_names[:n_params]] for m in in_maps]
    shard = NamedSharding(mesh, PartitionSpec("core"))
    concat_in = [
        jax.device_put(
            np.concatenate([per_core[c][i] for c in range(NCORES)], axis=0), shard
        )
        for i in range(n_params)
    ]
    concat_zeros = [
        jax.device_put(np.zeros((NCORES * z.shape[0], *z.shape[1:]), z.dtype), shard)
        for z in zero_outs
    ]
    outs = sharded(*concat_in, *concat_zeros)
    jax.block_until_ready(outs)
    best = None
    times = []
    for _ in range(n_iters):
        t0 = time.time()
        outs = sharded(*concat_in, *concat_zeros)
        jax.block_until_ready(outs)
        dt = time.time() - t0
        times.append(dt)
        best = dt if best is None else min(best, dt)
    print(f"warm iters (s): {[f'{x:.4f}' for x in times]}")

    oarr = np.asarray(outs[out_names.index("out")]).reshape(NCORES, BL, t_steps, V)
    full = _unpermute(list(oarr), perm, t_steps)
    return best, full
